# revision 24
# baseline (speedup 1.0000x reference)
"""Trainium2 Bass kernel for the dual-GRU-decoder ("Interpolation") problem.

Strategy (v2)
-------------
Two independent decoders (r: cells 1/2, p: cells 3/4), each a 64-step GRU
recurrence with B=2048, H=1024, D=128, n1=16.

The end-to-end run span is dominated by host<->device transfer over the
axon tunnel (~50 MB/s) and by per-call jit/lowering cost that scales with
the program's instruction count, not by device FLOPs. So v2 optimizes for
bytes shipped and program size:

* 2 cores, one decoder per core (no weight duplication; ~110 MB H2D total
  including the donated zero output buffers, vs ~280 MB for the 8-core
  data-parallel split).
* The 64 timesteps run in a hardware For_i loop (program is ~3k
  instructions instead of ~45k fully unrolled).
* Weights are streamed from HBM per 128-gate output chunk each step
  (19.7 MB/step, hidden under compute); SBUF holds the double-buffered
  hidden states for the full 2048 batch.
* bf16 weights/activations/outputs (tolerance is 2e-2; measured ~5e-3).

Layout is transposed throughout: feature/gate channels on partitions,
batch on the free dim, so no transposes happen on device.

Per step: x(t) = z_t (t<16, DMA'd under tc.If) or the previous out
(feedback copy); GRU1 reads h0_read, writes h0_new; GRU2 reads h1_read
(h-side) and h0_new (x-side), writes h1_new; out = w_out @ h1_new + b.
h*_new -> h*_read copy-backs at the end of the step keep the
read/write semantics of the recurrence explicit (no in-place aliasing).
"""

import time

import numpy as np
import ml_dtypes

BF16 = ml_dtypes.bfloat16
B, T, D, H, N1 = 2048, 64, 128, 1024, 16
TOUT = T - N1 + 1  # 49
HK = H // 128      # 8 hidden chunks
P = 128
NBT = B // 512     # 4 batch tiles of 512


_PROG = None
_TRACE = False
_last = {}


def _build_program():
    import concourse.mybir as mybir
    import concourse.tile as tile
    from concourse import bacc
    from concourse.bass import ds

    f32, bf16 = mybir.dt.float32, mybir.dt.bfloat16
    A = mybir.ActivationFunctionType
    E = mybir.EngineType
    nc = bacc.Bacc(None, target_bir_lowering=False)

    # Per-output-chunk weight slices: [o, K-row, k, gate-col]
    w1t = nc.dram_tensor("w1t", [24, P, 9, P], bf16, kind="ExternalInput")
    w2t = nc.dram_tensor("w2t", [24, P, 16, P], bf16, kind="ExternalInput")
    wot = nc.dram_tensor("wot", [HK, P, P], bf16, kind="ExternalInput")
    wit = nc.dram_tensor("wit", [P, H], bf16, kind="ExternalInput")
    bias = nc.dram_tensor("bias", [P, 73], f32, kind="ExternalInput")
    zt = nc.dram_tensor("zt", [N1, P, B], bf16, kind="ExternalInput")
    z8t = nc.dram_tensor("z8t", [P, B], bf16, kind="ExternalInput")
    out_d = nc.dram_tensor("out", [TOUT, P, B], bf16, kind="ExternalOutput")

    with tile.TileContext(nc) as tc:
        with (
            tc.tile_pool(name="res", bufs=1) as rpool,
            tc.tile_pool(name="st", bufs=1) as spool,
            tc.tile_pool(name="w1s", bufs=5) as w1pool,
            tc.tile_pool(name="w2s", bufs=5) as w2pool,
            tc.tile_pool(name="rz", bufs=4) as rzpool,
            tc.tile_pool(name="tmp", bufs=4) as tpool,
            tc.tile_pool(name="psum", bufs=8, space="PSUM") as ppool,
        ):
            # ---- small resident tensors ----
            wo = rpool.tile([P, HK, P], bf16, tag="wo")
            nc.sync.dma_start(wo[:], wot.rearrange("o p f -> p o f"))
            bia = rpool.tile([P, 73], f32, tag="bias")
            nc.sync.dma_start(bia[:], bias[:])
            brz1, bni1, bnh1 = bia[:, 0:16], bia[:, 16:24], bia[:, 24:32]
            brz2, bni2, bnh2 = bia[:, 32:48], bia[:, 48:56], bia[:, 56:64]
            bout, bini = bia[:, 64:65], bia[:, 65:73]
            witl = rpool.tile([P, H], bf16, tag="wit")
            nc.sync.dma_start(witl[:], wit[:])
            z8l = rpool.tile([P, B], bf16, tag="z8")
            nc.sync.dma_start(z8l[:], z8t[:])

            # ---- state ----
            h0r = spool.tile([P, HK, B], bf16, tag="h0r", name="h0r")
            h0n = spool.tile([P, HK, B], bf16, tag="h0n", name="h0n")
            h1r = spool.tile([P, HK, B], bf16, tag="h1r", name="h1r")
            h1n = spool.tile([P, HK, B], bf16, tag="h1n", name="h1n")
            xbuf = spool.tile([P, B], bf16, tag="xbuf", name="xbuf")
            outw = spool.tile([P, B], bf16, tag="outw", name="outw")

            tc.strict_bb_all_engine_barrier()

            # ---- h0 init: h0 = z8 @ w_init.T + b_init ----
            for m in range(HK):
                for b in range(NBT):
                    ps = ppool.tile([P, 512], f32, tag="acc")
                    nc.tensor.matmul(ps[:], witl[:, ds(m * P, P)],
                                     z8l[:, ds(b * 512, 512)],
                                     start=True, stop=True)
                    nc.scalar.activation(h0r[:, m, ds(b * 512, 512)], ps[:],
                                         A.Identity, bias=bini[:, m:m + 1])

            tc.strict_bb_all_engine_barrier()

            def gru_cell(wt, nk, h_side, x_side, brz, bni, bnh, h_write):
                """One GRU cell sweep over all 8 output chunks x 4 batch tiles.

                wt: DRAM weight tensor [24, P, nk, P]; h_side/x_side: lists of
                (k, sbuf_chunk_fn) contraction inputs for the h-part / x-part.
                """
                for ih in range(HK):
                    ws = []
                    for g in range(3):  # r, z, n slices
                        w = (w1pool if nk == 9 else w2pool).tile(
                            [P, nk, P], bf16, tag=f"w{nk}s")
                        # alternate DMA-trigger engines so the weight stream
                        # uses two queues in parallel
                        eng = nc.sync if (g * 8 + ih) % 2 == 0 else nc.gpsimd
                        eng.dma_start(w[:], wt[g * 8 + ih])
                        ws.append(w)
                    wr, wz, wn = ws
                    for b in range(NBT):
                        bsl = ds(b * 512, 512)
                        pr = ppool.tile([P, 512], f32, tag="acc")
                        pz = ppool.tile([P, 512], f32, tag="acc")
                        phn = ppool.tile([P, 512], f32, tag="acc")
                        pin = ppool.tile([P, 512], f32, tag="acc")
                        nrz = len(h_side) + len(x_side)
                        for pp, w in ((pr, wr), (pz, wz)):
                            j = 0
                            for k, src in h_side:
                                nc.tensor.matmul(pp[:], w[:, k, :], src(bsl),
                                                 start=(j == 0), stop=(j == nrz - 1))
                                j += 1
                            for k, src in x_side:
                                nc.tensor.matmul(pp[:], w[:, k, :], src(bsl),
                                                 start=(j == 0), stop=(j == nrz - 1))
                                j += 1
                        for j, (k, src) in enumerate(h_side):
                            nc.tensor.matmul(phn[:], wn[:, k, :], src(bsl),
                                             start=(j == 0), stop=(j == len(h_side) - 1))
                        for j, (k, src) in enumerate(x_side):
                            nc.tensor.matmul(pin[:], wn[:, k, :], src(bsl),
                                             start=(j == 0), stop=(j == len(x_side) - 1))
                        r = rzpool.tile([P, 512], bf16, tag="r")
                        zz = rzpool.tile([P, 512], bf16, tag="z")
                        nc.scalar.activation(r[:], pr[:], A.Sigmoid,
                                             bias=brz[:, ih:ih + 1])
                        nc.scalar.activation(zz[:], pz[:], A.Sigmoid,
                                             bias=brz[:, HK + ih:HK + ih + 1])
                        a = tpool.tile([P, 512], f32, tag="tmp")
                        nt = tpool.tile([P, 512], f32, tag="tmp")
                        nc.scalar.add(a[:], phn[:], bnh[:, ih:ih + 1])
                        nc.vector.tensor_mul(a[:], r[:], a[:])
                        nc.vector.tensor_add(a[:], a[:], pin[:])
                        nc.scalar.activation(nt[:], a[:], A.Tanh,
                                             bias=bni[:, ih:ih + 1])
                        h_old = (h0r if h_write is h0n else h1r)
                        nc.vector.tensor_sub(a[:], h_old[:, ih, bsl], nt[:])
                        nc.vector.tensor_mul(a[:], zz[:], a[:])
                        nc.vector.tensor_add(h_write[:, ih, bsl], nt[:], a[:])

            h0r_src = [(1 + k, (lambda k=k: lambda bsl: h0r[:, k, bsl])())
                       for k in range(HK)]
            x_src = [(0, lambda bsl: xbuf[:, bsl])]
            h1r_src = [(8 + k, (lambda k=k: lambda bsl: h1r[:, k, bsl])())
                       for k in range(HK)]
            h0n_src = [(k, (lambda k=k: lambda bsl: h0n[:, k, bsl])())
                       for k in range(HK)]

            with tc.For_i(0, T, hint_engines=(E.PE, E.DVE, E.Activation)) as i:
                with tc.If(i < N1):
                    iw = nc.s_assert_within(i, 0, N1 - 1, skip_runtime_assert=True)
                    nc.sync.dma_start(xbuf[:], zt[ds(iw, 1)])

                gru_cell(w1t, 9, h0r_src, x_src, brz1, bni1, bnh1, h0n)

                with tc.If(i < 1):
                    nc.vector.tensor_copy(h1r[:], h0n[:])

                gru_cell(w2t, 16, h1r_src, h0n_src, brz2, bni2, bnh2, h1n)

                # out = h1n @ w_out.T + b_out
                for b in range(NBT):
                    bsl = ds(b * 512, 512)
                    po = ppool.tile([P, 512], f32, tag="acc")
                    for k in range(HK):
                        nc.tensor.matmul(po[:], wo[:, k, :], h1n[:, k, bsl],
                                         start=(k == 0), stop=(k == HK - 1))
                    nc.scalar.activation(outw[:, bsl], po[:], A.Identity,
                                         bias=bout[:, 0:1])

                with tc.If(i >= N1 - 1):
                    io = nc.s_assert_within(i - (N1 - 1), 0, TOUT - 1,
                                            skip_runtime_assert=True)
                    nc.sync.dma_start(out_d[ds(io, 1)], outw[:])

                # state copy-backs + autoregressive feedback for the next step
                nc.vector.tensor_copy(xbuf[:], outw[:])
                nc.vector.tensor_copy(h0r[:], h0n[:])
                nc.vector.tensor_copy(h1r[:], h1n[:])
    nc.finalize()
    return nc


def _get_prog():
    global _PROG
    if _PROG is None:
        _PROG = _build_program()
    return _PROG


def _chunked(wcat, nk):
    # [nk*128, 3072] -> [24, 128, nk, 128] per-output-chunk slices
    return np.ascontiguousarray(
        wcat.reshape(nk, P, 24, P).transpose(2, 1, 0, 3)).astype(BF16)


def _prep_core(z, z8, wi1, wh1, bi1, bh1, wi2, wh2, bi2, bh2,
               w_init, b_init, w_out, b_out):
    f32 = np.float32
    w1t = _chunked(np.concatenate([wi1.T, wh1.T], 0), 9)
    w2t = _chunked(np.concatenate([wi2.T, wh2.T], 0), 16)
    wot = np.ascontiguousarray(w_out.T).astype(BF16).reshape(HK, P, P)
    wit = np.ascontiguousarray(w_init.T).astype(BF16)
    bias = np.zeros((P, 73), f32)
    bias[:, 0:16] = (bi1 + bh1)[:2048].reshape(16, P).T
    bias[:, 16:24] = bi1[2048:].reshape(8, P).T
    bias[:, 24:32] = bh1[2048:].reshape(8, P).T
    bias[:, 32:48] = (bi2 + bh2)[:2048].reshape(16, P).T
    bias[:, 48:56] = bi2[2048:].reshape(8, P).T
    bias[:, 56:64] = bh2[2048:].reshape(8, P).T
    bias[:, 64] = b_out
    bias[:, 65:73] = b_init.reshape(8, P).T
    ztp = np.ascontiguousarray(z[:, :N1, :].transpose(1, 2, 0)).astype(BF16)
    z8tp = np.ascontiguousarray(z8.T).astype(BF16)
    return dict(w1t=w1t, w2t=w2t, wot=wot, wit=wit,
                bias=np.ascontiguousarray(bias), zt=ztp, z8t=z8tp)


def kernel(**inputs):
    n1 = int(inputs.get("n1", 16))
    assert n1 == N1, f"kernel hardcodes n1={N1}, got {n1}"
    g = {k: np.asarray(v, dtype=np.float32) if k not in ("n1", "n2") else v
         for k, v in inputs.items()}

    in_maps = [
        _prep_core(g["zr"], g["zr8"],
                   g["wi1"], g["wh1"], g["bi1"], g["bh1"],
                   g["wi2"], g["wh2"], g["bi2"], g["bh2"],
                   g["w_init0"], g["b_init0"], g["w_out0"], g["b_out0"]),
        _prep_core(g["zp"], g["zp8"],
                   g["wi3"], g["wh3"], g["bi3"], g["bh3"],
                   g["wi4"], g["wh4"], g["bi4"], g["bh4"],
                   g["w_init1"], g["b_init1"], g["w_out1"], g["b_out1"]),
    ]

    from concourse.bass_utils import run_bass_kernel_spmd
    nc = _get_prog()
    t0 = time.time()
    res = run_bass_kernel_spmd(nc, in_maps, core_ids=[0, 1], trace=_TRACE)
    _last["run_s"] = time.time() - t0
    _last["exec_time_ns"] = res.exec_time_ns
    _last["trace"] = res.instructions_and_trace
    outs = [np.asarray(r["out"]).astype(np.float32).transpose(2, 0, 1)
            for r in res.results]
    return outs[1], outs[0]  # (z_p, z_r)
